# revision 63
# baseline (speedup 1.0000x reference)
"""Trainium2 Bass kernel for nn_CrossFeature (sparse_attention).

Math (per batch b):
    att[b,n,f]  = (x[b] @ W.T @ q.T).T * E**-0.5          # folded: x[b] @ (qW).T
    Xs          = 0.5 * att                               # entmax15 pre-scale
    gate        = entmax15(att) over f  (solved by Newton on the entmax root)
    out[b,n,e]  = exp( sum_f gate*value * x[b,f,e] )

Key algebraic moves:
  * stage-1/2 fused: qtilde = (q @ W) * 0.5 * E**-0.5, Xs = x @ qtilde.T
  * entmax15 bisection (50 iters) replaced by Newton on
        g(tau) = sum_f relu(Xs-tau)^2 - 1,
    with moments from bn_stats over m = max(Xs, tau):
        s1 = sum relu(Xs-tau)   = 32*((mean_e-tau)+(mean_o-tau))
        s2 = sum relu(Xs-tau)^2 = M2_e + M2_o + 32*((mean_e-tau)^2+(mean_o-tau)^2)
    init tau0 = mean - (cbar/2 + (1 - v64)/(128*cbar))  (linearized sqrt)

Dataflow (v5i, 166us; baseline v4 was 206us):
  * x loaded ONCE, cast fp32->bf16 inside the DMA (SWDGE on GpSimd).
  * x^T via PE-array transposes (bf16) -> PSUM -> scalar/vector evac
    (evac copies split between scalar+vector to balance engine load).
  * stage-12 BATCHED: one matmul per (8-pair blk, e-chunk, row-half) with a
    strided 3D rhs AP (FD=512) instead of 8 FD=64 matmuls.
  * Xs kept in bf16; tau duplicated into bf16 pairs so the broadcast
    max/sub TT ops hit the DVE 2x_1P packed mode (innermost step 1).
  * entmax moments via per-c bn_stats (fine-grained ops interleave across
    the two lanes far better than coarse tensor_reduces -- measured).
  * NO final normalization: the Newton step drives sum(relu(Xs-tau)^2)->1
    to second order (|s2-1|~2e-3, below bf16 noise), so aw = d^2*v
    directly -- no reduce, no reciprocal, no renorm multiply.
  * stage-3 fully bf16; Exp batched FD=1024 (2-bank PSUM tiles); output
    stored as bf16 and upcast to fp32 on the host.
  * groups processed TWO AT A TIME with their instruction streams
    interleaved op-by-op (per-engine queues are strict FIFO).
  Rejected experimentally: DMA x-bar transposes (serialize ~1.2us/block on
  the HWDGE path -> 3.6x slower), free-running scheduler (phase convoys),
  4-wide lanes at C=8 (per-op overhead), split-batch aw transposes.

Sharding: pure data-parallel, batch 2048 -> 8 cores x 256.
"""

import numpy as np

B_FULL, F, E, N = 2048, 64, 256, 64
NCORES = 8
B_LOC = B_FULL // NCORES

SCALE = 0.5 * (E ** -0.5)   # folds entmax's (alpha-1) into qtilde
CBAR = 0.097                # linearization point for sqrt((1-v64)/64)


def build_program(B_loc=B_LOC, NG=8):
    import concourse.tile as tile
    from concourse import bacc, mybir, masks

    f32 = mybir.dt.float32
    bf16 = mybir.dt.bfloat16
    Alu = mybir.AluOpType
    ACTF = mybir.ActivationFunctionType

    HALF = B_loc // 2
    C = HALF // NG            # batch-pairs per group
    half = C // 2
    assert C * NG == HALF and C % 8 == 0 and NG % 2 == 0

    nc = bacc.Bacc("TRN2", debug=False, num_devices=NCORES)
    x_d = nc.dram_tensor("x", [B_loc, F, E], f32, kind="ExternalInput").ap()
    w_d = nc.dram_tensor("bilinear_w", [E, E], f32, kind="ExternalInput").ap()
    q_d = nc.dram_tensor("query", [N, E], f32, kind="ExternalInput").ap()
    v_d = nc.dram_tensor("value", [N, F], f32, kind="ExternalInput").ap()
    o_d = nc.dram_tensor("out", [B_loc, N, E], bf16, kind="ExternalOutput").ap()

    K0 = 0.5 * CBAR + 1.0 / (128.0 * CBAR)
    KW = 1.0 / (128.0 * CBAR)

    with tile.TileContext(nc) as tc:
        with (
            tc.tile_pool(name="const", bufs=1) as constp,
            tc.tile_pool(name="xbf", bufs=7) as xbfp,
            tc.tile_pool(name="xtg", bufs=3) as xtgp,
            tc.tile_pool(name="xs", bufs=4) as xsp,
            tc.tile_pool(name="mb", bufs=4) as mbp,
            tc.tile_pool(name="dbf", bufs=8) as dbfp,
            tc.tile_pool(name="st", bufs=4) as stp,
            tc.tile_pool(name="sm", bufs=4) as smp,
            tc.tile_pool(name="awt", bufs=2) as awtp,
            tc.tile_pool(name="osb", bufs=3) as osbp,
            tc.tile_pool(name="pstx", bufs=2, space="PSUM") as pstxp,
            tc.tile_pool(name="ps12", bufs=1, space="PSUM") as ps12p,
            tc.tile_pool(name="ps3", bufs=2, space="PSUM") as ps3p,
            tc.tile_pool(name="psaw", bufs=1, space="PSUM") as psawp,
        ):
            # ---------------- constants ----------------
            ident = constp.tile([128, 128], f32)
            masks.make_identity(nc, ident[:])
            ident_bf = constp.tile([128, 128], bf16, tag="identbf")
            nc.gpsimd.tensor_copy(ident_bf[:], ident[:])

            v2 = constp.tile([128, F], f32)
            nc.sync.dma_start(v2[0:64, :], v_d[:, :])
            nc.sync.dma_start(v2[64:128, :], v_d[:, :])
            v2bf = constp.tile([128, F], bf16, tag="v2bf")
            nc.gpsimd.tensor_copy(v2bf[:], v2[:])

            wt = {}
            for di in range(2):
                for ej in range(2):
                    t = constp.tile([128, 128], f32, tag=f"wt{di}{ej}")
                    nc.sync.dma_start(
                        t[:], w_d[di * 128:(di + 1) * 128, ej * 128:(ej + 1) * 128]
                    )
                    wt[di, ej] = t

            # load q naturally (contiguous DMA) and transpose on the PE —
            # a transposed DMA access pattern on q generates 8K 4-byte
            # descriptors and stalls the sync queue for ~25us.
            qn = constp.tile([64, E], f32, tag="qn")
            nc.sync.dma_start(qn[:], q_d[:, :])
            psq = ps12p.tile([128, 512], f32, tag="ps12")
            qtin = []
            for di in range(2):
                nc.tensor.transpose(
                    psq[:, di * 64:(di + 1) * 64],
                    qn[:, di * 128:(di + 1) * 128],
                    ident[0:64, 0:64],
                )
                t = constp.tile([128, N], f32, tag=f"qtin{di}")
                nc.scalar.copy(t[:], psq[:, di * 64:(di + 1) * 64])
                qtin.append(t)

            # qtilde^T = W.T-contract: qt[e, n] = sum_d W[d, e] q[n, d], then * SCALE
            qt_bf = []
            for ej in range(2):
                ps = ps12p.tile([128, 512], f32, tag="ps12")
                for di in range(2):
                    nc.tensor.matmul(
                        ps[:, 0:N], wt[di, ej][:], qtin[di][:],
                        start=(di == 0), stop=(di == 1),
                    )
                t = constp.tile([128, N], bf16, tag=f"qtbf{ej}")
                nc.scalar.mul(t[:], ps[:, 0:N], SCALE)
                qt_bf.append(t)

            # ---------------- per-group emit helpers ----------------
            def emit_front(g):
                """Generator: loads + PE transposes + stage-12 for group g.
                Returns (xbf, xs_t) via StopIteration."""
                b0 = g * C
                xbf = xbfp.tile([128, C, E], bf16, tag="xbf")
                for q in range(C // 8):
                    s0 = q * 8
                    nc.gpsimd.dma_start(
                        xbf[0:64, s0:s0 + 8, :],
                        x_d[b0 + s0:b0 + s0 + 8, :, :].transpose([1, 0, 2]),
                    )
                    nc.gpsimd.dma_start(
                        xbf[64:128, s0:s0 + 8, :],
                        x_d[HALF + b0 + s0:HALF + b0 + s0 + 8, :, :]
                        .transpose([1, 0, 2]),
                    )
                    yield

                # x^T via PE transposes (the DMA x-bar path serializes at
                # ~1.2us per 128x128 block on the HWDGE engine -- measured
                # 3.6x worse end-to-end).
                xtg = xtgp.tile([128, C, 2, 128], bf16, tag="xtg")
                for c4 in range(C // 4):
                    pst = pstxp.tile([128, 4, 2, 128], bf16, tag="pstx")
                    for k in range(4):
                        c = c4 * 4 + k
                        for ec in range(2):
                            nc.tensor.transpose(
                                pst[:, k, ec, :],
                                xbf[:, c, ec * 128:(ec + 1) * 128],
                                ident_bf[:],
                            )
                    # balance PSUM evac load: scalar is the busiest engine,
                    # vector has headroom -- split the copies between them.
                    if c4 == 0:
                        nc.vector.tensor_copy(
                            xtg[:, c4 * 4:c4 * 4 + 4, :, :], pst[:]
                        )
                    else:
                        nc.scalar.copy(xtg[:, c4 * 4:c4 * 4 + 4, :, :], pst[:])
                    yield

                xs_t = xsp.tile([128, C, 72], bf16, tag="xs")
                xs3 = xs_t[:, :, 0:F]
                for blk in range(C // 8):
                    ps = ps12p.tile([128, 512], f32, tag="ps12")
                    psA = ps[0:64, :].rearrange("p (c f) -> p c f", f=64)
                    psB = ps[64:128, :].rearrange("p (c f) -> p c f", f=64)
                    c0 = blk * 8
                    for ec in range(2):
                        # one FD=512 matmul per row-half covering 8 pairs
                        # (strided 3D rhs AP); start=True only on the first
                        # MM of each row-half (clears has_written for the
                        # MM's rows across the whole bank).
                        nc.tensor.matmul(
                            psA, qt_bf[ec][:],
                            xtg[:, c0:c0 + 8, ec, 0:64],
                            start=(ec == 0), stop=(ec == 1),
                            tile_position=(0, 0),
                            skip_group_check=True,
                        )
                        nc.tensor.matmul(
                            psB, qt_bf[ec][:],
                            xtg[:, c0:c0 + 8, ec, 64:128],
                            start=(ec == 0), stop=(ec == 1),
                            tile_position=(0, 64),
                            skip_group_check=True,
                        )
                        yield
                    nc.vector.tensor_copy(
                        xs3[:, c0:c0 + 8, :],
                        ps[:].rearrange("p (c f) -> p c f", f=F),
                    )
                    yield
                return xbf, xs_t

            def emit_entmax(xs_t):
                """Generator: entmax Newton solve, bf16 data-path with tau
                duplicated into bf16 pairs so the broadcast TT ops run in
                the DVE 2x mode.  Moments via per-c bn_stats (fine-grained
                ops interleave across lanes better than coarse reduces)."""
                xs3 = xs_t[:, :, 0:F]
                xs4 = xs3.rearrange("p c (k two) -> p c k two", two=2)
                st = stp.tile([128, C, 6], f32, tag="st")

                def sl(k):
                    return st[:, :, k:k + 1]        # [128, C, 1]

                tau = smp.tile([128, C], f32, tag="tau")
                tauu = tau[:].unsqueeze(2)
                tau2 = smp.tile([128, C, 2], bf16, tag="tau2")
                tau4 = tau2[:].unsqueeze(2).broadcast_to([128, C, 32, 2])
                s2s = smp.tile([128, C], f32, tag="s2s")
                rcp = smp.tile([128, C], f32, tag="rcp")

                # ---- init moments + tau0 ----
                for c in range(C):
                    nc.vector.bn_stats(st[:, c, :], xs3[:, c, :])
                    yield
                msum = smp.tile([128, C], f32, tag="msum")
                wsum = smp.tile([128, C], f32, tag="wsum")
                nc.vector.tensor_add(msum[:].unsqueeze(2), sl(1), sl(4))
                yield
                nc.vector.tensor_add(wsum[:].unsqueeze(2), sl(2), sl(5))
                yield
                nc.vector.tensor_scalar(
                    out=msum[:], in0=msum[:], scalar1=0.5, scalar2=K0,
                    op0=Alu.mult, op1=Alu.subtract,
                )
                yield
                nc.vector.scalar_tensor_tensor(
                    out=tau[:], in0=wsum[:], scalar=KW, in1=msum[:],
                    op0=Alu.mult, op1=Alu.add,
                )
                yield
                nc.vector.tensor_copy(
                    tau2[:], tau[:].unsqueeze(2).broadcast_to([128, C, 2])
                )
                yield

                mb_t = mbp.tile([128, C, 72], bf16, tag="mb")
                mb3 = mb_t[:, :, 0:F]
                mb4 = mb3.rearrange("p c (k two) -> p c k two", two=2)

                a2 = smp.tile([128, C, 2], f32, tag="a2")
                u2 = smp.tile([128, C, 2], f32, tag="u2")
                s1m = smp.tile([128, C], f32, tag="s1m")

                # ---- one Newton iteration (moments via bn_stats) ----
                nc.vector.tensor_max(mb4, xs4, tau4)
                yield
                for c in range(C):
                    nc.vector.bn_stats(st[:, c, :], mb3[:, c, :])
                    yield
                nc.vector.tensor_sub(a2[:, :, 0:1], sl(1), tauu)
                yield
                nc.vector.tensor_sub(a2[:, :, 1:2], sl(4), tauu)
                yield
                nc.vector.tensor_mul(u2[:], a2[:], a2[:])
                yield
                nc.vector.scalar_tensor_tensor(
                    out=u2[:, :, 0:1], in0=u2[:, :, 0:1], scalar=32.0,
                    in1=sl(2), op0=Alu.mult, op1=Alu.add,
                )
                yield
                nc.vector.scalar_tensor_tensor(
                    out=u2[:, :, 1:2], in0=u2[:, :, 1:2], scalar=32.0,
                    in1=sl(5), op0=Alu.mult, op1=Alu.add,
                )
                yield
                nc.vector.tensor_reduce(
                    s2s[:], u2[:], axis=mybir.AxisListType.X, op=Alu.add,
                )
                yield
                nc.vector.tensor_reduce(
                    s1m[:], a2[:], axis=mybir.AxisListType.X, op=Alu.add,
                )
                yield
                nc.vector.reciprocal_approx_fast(rcp[:], s1m[:])
                yield
                nc.vector.tensor_scalar(
                    out=s2s[:], in0=s2s[:], scalar1=-1.0, scalar2=None,
                    op0=Alu.add,
                )
                yield
                nc.vector.tensor_mul(s2s[:], s2s[:], rcp[:])
                yield
                nc.vector.scalar_tensor_tensor(
                    out=tau[:], in0=s2s[:], scalar=1.0 / 64.0, in1=tau[:],
                    op0=Alu.mult, op1=Alu.add,
                )
                yield
                nc.vector.tensor_copy(
                    tau2[:], tau[:].unsqueeze(2).broadcast_to([128, C, 2])
                )
                yield

                # ---- final eval ----
                # No final normalization: the Newton update drives s2 -> 1
                # to second order (|s2-1| ~ 2e-3, below bf16 noise), so
                # aw = relu(Xs-tau)^2 * v directly.
                nc.vector.tensor_max(mb4, xs4, tau4)
                yield
                d_bf = dbfp.tile([128, C, F], bf16, tag="dbf")
                d4 = d_bf[:].rearrange("p c (k two) -> p c k two", two=2)
                nc.vector.tensor_sub(d4, mb4, tau4)
                yield
                s_bf = dbfp.tile([128, C, F], bf16, tag="sbf")
                nc.vector.tensor_mul(s_bf[:], d_bf[:], d_bf[:])
                yield
                t_bf = dbfp.tile([128, C, F], bf16, tag="tbf")
                nc.vector.tensor_mul(
                    t_bf[:], s_bf[:],
                    v2bf[:].unsqueeze(1).broadcast_to([128, C, F]),
                )
                yield
                return t_bf

            def emit_back(g, xbf, t_bf):
                """Generator: PE transposes of aw + stage-3 + Exp + stores."""
                b0 = g * C
                aw_t = t_bf[:].rearrange("p c f -> p (c f)")

                # aw^T: bf16 PE transposes + evac; one shift DMA/group
                awt_g = awtp.tile([128, C, 64], bf16, tag="awt")
                awt_tmp = awtp.tile([64, C, 64], bf16, tag="awt_tmp")
                for hb in range(2):
                    pst = psawp.tile([64, half, 128], bf16, tag="psaw")
                    for s in range(half):
                        c = hb * half + s
                        nc.tensor.transpose(
                            pst[:, s, :], aw_t[:, c * F:(c + 1) * F],
                            ident_bf[:],
                        )
                        if s % 4 == 3:
                            yield
                    nc.scalar.copy(
                        awt_g[0:64, hb * half:(hb + 1) * half, :],
                        pst[:, :, 0:64],
                    )
                    nc.scalar.copy(
                        awt_tmp[:, hb * half:(hb + 1) * half, :],
                        pst[:, :, 64:128],
                    )
                    yield
                nc.sync.dma_start(awt_g[64:128, :, :], awt_tmp[:])
                yield

                # stage-3 (bf16): out = exp(awt.T @ x); Exp batched FD=1024
                for hb in range(2):
                    osb = osbp.tile([128, half, E], bf16, tag="osb")
                    for qq in range(C // 8):
                        ps3 = ps3p.tile([128, 1024], f32, tag="ps3")
                        # A/B matmuls interleaved per pair: adjacent A and B
                        # run concurrently in disjoint PE quadrants
                        # ((0,0) vs (64,64)) -- grouping all-A-then-all-B
                        # loses that packing (measured +18us).
                        for si in range(4):
                            c = hb * half + qq * 4 + si
                            nc.tensor.matmul(
                                ps3[0:64, si * 256:(si + 1) * 256],
                                awt_g[0:64, c, :],
                                xbf[0:64, c, :],
                                start=True, stop=True,
                                tile_position=(0, 0),
                                skip_group_check=True,
                            )
                            nc.tensor.matmul(
                                ps3[64:128, si * 256:(si + 1) * 256],
                                awt_g[64:128, c, :],
                                xbf[64:128, c, :],
                                start=True, stop=True,
                                tile_position=(64, 64),
                                skip_group_check=True,
                            )
                        nc.scalar.activation(
                            osb[:, qq * 4:(qq + 1) * 4, :],
                            ps3[:].rearrange("p (c e) -> p c e", e=E),
                            ACTF.Exp,
                        )
                        yield
                    bA = b0 + hb * half
                    nc.sync.dma_start(
                        o_d[bA:bA + half, :, :].transpose([1, 0, 2]),
                        osb[0:64, :, :],
                    )
                    nc.sync.dma_start(
                        o_d[HALF + bA:HALF + bA + half, :, :]
                        .transpose([1, 0, 2]),
                        osb[64:128, :, :],
                    )
                    yield

            def drive(items, carry, conts):
                """Round-robin generators.  `items` = [(gen, key)] that must
                finish this epoch; `carry` = keyless generators kept running;
                `conts[key](result)` spawns a follow-on generator (appended,
                keyless).  Returns (results, leftover carry generators)."""
                res = {}
                must = {k for _, k in items}
                act = [[g, k] for g, k in items] + [[g, None] for g in carry]
                while must - res.keys():
                    for job in list(act):
                        gen, key = job
                        try:
                            next(gen)
                        except StopIteration as e:
                            if key is not None:
                                res[key] = e.value
                            act.remove(job)
                            if key in conts:
                                act.append([conts[key](e.value), None])
                return res, [g for g, _ in act]

            # 3-stage pipeline, LANES-wide: epoch k interleaves
            # front(k+1-lane-set) | entmax(k-lane-set) | leftover backs;
            # each back spawns the moment its entmax finishes.  Wider lanes
            # overlap more per-group serial dependency chains (the span is
            # latency-bound, not engine-throughput-bound).
            LANES = 2
            NS = NG // LANES
            res, carry = drive(
                [(emit_front(j), f'f{j}') for j in range(LANES)], [], {})
            fr = [res[f'f{j}'] for j in range(LANES)]
            for k in range(NS):
                items = [(emit_entmax(fr[j][1]), f'e{j}')
                         for j in range(LANES)]
                if k + 1 < NS:
                    items += [(emit_front(LANES * (k + 1) + j), f'f{j}')
                              for j in range(LANES)]
                conts = {
                    f'e{j}': (lambda r, xb=fr[j][0], g=LANES * k + j:
                              emit_back(g, xb, r))
                    for j in range(LANES)
                }
                res, carry = drive(items, carry, conts)
                if k + 1 < NS:
                    fr = [res[f'f{j}'] for j in range(LANES)]
            # drain the remaining backs
            drive([(g, i) for i, g in enumerate(carry)], [], {})
    if not nc.is_finalized():
        nc.finalize()
    return nc


_NC_CACHE = {}


def _get_program(B_loc, NG):
    key = (B_loc, NG)
    if key not in _NC_CACHE:
        _NC_CACHE[key] = build_program(B_loc, NG)
    return _NC_CACHE[key]


def kernel(**inputs):
    from concourse.bass_utils import run_bass_kernel_spmd

    x = np.ascontiguousarray(np.asarray(inputs["x"], dtype=np.float32))
    w = np.ascontiguousarray(np.asarray(inputs["bilinear_w"], dtype=np.float32))
    q = np.ascontiguousarray(np.asarray(inputs["query"], dtype=np.float32))
    v = np.ascontiguousarray(np.asarray(inputs["value"], dtype=np.float32))
    B = x.shape[0]
    B_loc = B // NCORES

    nc = _get_program(B_loc, 8)

    in_maps = []
    for core in range(NCORES):
        sh = x[core * B_loc:(core + 1) * B_loc]
        in_maps.append(
            {"x": np.ascontiguousarray(sh), "bilinear_w": w, "query": q, "value": v}
        )

    import os
    trace = bool(int(os.environ.get("KERNEL_TRACE", "0")))
    res = run_bass_kernel_spmd(
        nc, in_maps, core_ids=list(range(NCORES)), trace=trace,
        trace_cores=[0] if trace else None,
    )
    if trace:
        kernel.last_exec_time_ns = res.exec_time_ns
        kernel.last_trace = res.instructions_and_trace
    out = np.concatenate([r["out"] for r in res.results], axis=0)
    return out.astype(np.float32)


if __name__ == "__main__":
    # smoke-test the builder only
    nc = build_program(32, 2)
    print("build ok:", len(nc.inst_map), "instructions")


# revision 65
# speedup vs baseline: 1.0483x; 1.0483x over previous
"""Trainium2 Bass kernel for nn_CrossFeature (sparse_attention).

Math (per batch b):
    att[b,n,f]  = (x[b] @ W.T @ q.T).T * E**-0.5          # folded: x[b] @ (qW).T
    Xs          = 0.5 * att                               # entmax15 pre-scale
    gate        = entmax15(att) over f  (solved by Newton on the entmax root)
    out[b,n,e]  = exp( sum_f gate*value * x[b,f,e] )

Key algebraic moves:
  * stage-1/2 fused: qtilde = (q @ W) * 0.5 * E**-0.5, Xs = x @ qtilde.T
  * entmax15 bisection (50 iters) replaced by Newton on
        g(tau) = sum_f relu(Xs-tau)^2 - 1,
    with moments from bn_stats over m = max(Xs, tau):
        s1 = sum relu(Xs-tau)   = 32*((mean_e-tau)+(mean_o-tau))
        s2 = sum relu(Xs-tau)^2 = M2_e + M2_o + 32*((mean_e-tau)^2+(mean_o-tau)^2)
    init tau0 = mean - (cbar/2 + (1 - v64)/(128*cbar))  (linearized sqrt)

Dataflow (v6d, ~155us; baseline v4 was 206us).  Pool buffer counts are
load-bearing: xbf=7/dbf=8/osb=3 removed ~12us of pool-reuse (WAR) stalls
vs xbf=6/dbf=7/osb=2; pushing further (xtg=4/xs=5/mb=5/awt=3) regressed.
  * x loaded ONCE, cast fp32->bf16 inside the DMA (SWDGE on GpSimd).
  * x^T via PE-array transposes (bf16) -> PSUM -> scalar/vector evac
    (evac copies split between scalar+vector to balance engine load).
  * stage-12 BATCHED: one matmul per (8-pair blk, e-chunk, row-half) with a
    strided 3D rhs AP (FD=512) instead of 8 FD=64 matmuls.
  * Xs kept in bf16; tau duplicated into bf16 pairs so the broadcast
    max/sub TT ops hit the DVE 2x_1P packed mode (innermost step 1).
  * entmax moments via per-c bn_stats (fine-grained ops interleave across
    the two lanes far better than coarse tensor_reduces -- measured).
  * NO final normalization: the Newton step drives sum(relu(Xs-tau)^2)->1
    to second order (|s2-1|~2e-3, below bf16 noise), so aw = d^2*v
    directly -- no reduce, no reciprocal, no renorm multiply.
  * stage-3 fully bf16; Exp batched FD=1024 (2-bank PSUM tiles); output
    stored as bf16 and upcast to fp32 on the host.
  * groups processed TWO AT A TIME with their instruction streams
    interleaved op-by-op (per-engine queues are strict FIFO).
  Rejected experimentally: DMA x-bar transposes (serialize ~1.2us/block on
  the HWDGE path -> 3.6x slower), free-running scheduler (phase convoys),
  4-wide lanes at C=8 (per-op overhead), split-batch aw transposes.

Sharding: pure data-parallel, batch 2048 -> 8 cores x 256.
"""

import numpy as np

B_FULL, F, E, N = 2048, 64, 256, 64
NCORES = 8
B_LOC = B_FULL // NCORES

SCALE = 0.5 * (E ** -0.5)   # folds entmax's (alpha-1) into qtilde
CBAR = 0.097                # linearization point for sqrt((1-v64)/64)


def build_program(B_loc=B_LOC, NG=8):
    import concourse.tile as tile
    from concourse import bacc, mybir, masks

    f32 = mybir.dt.float32
    bf16 = mybir.dt.bfloat16
    Alu = mybir.AluOpType
    ACTF = mybir.ActivationFunctionType

    HALF = B_loc // 2
    C = HALF // NG            # batch-pairs per group
    half = C // 2
    assert C * NG == HALF and C % 8 == 0 and NG % 2 == 0

    nc = bacc.Bacc("TRN2", debug=False, num_devices=NCORES)
    x_d = nc.dram_tensor("x", [B_loc, F, E], f32, kind="ExternalInput").ap()
    w_d = nc.dram_tensor("bilinear_w", [E, E], f32, kind="ExternalInput").ap()
    q_d = nc.dram_tensor("query", [N, E], f32, kind="ExternalInput").ap()
    v_d = nc.dram_tensor("value", [N, F], f32, kind="ExternalInput").ap()
    o_d = nc.dram_tensor("out", [B_loc, N, E], bf16, kind="ExternalOutput").ap()

    K0 = 0.5 * CBAR + 1.0 / (128.0 * CBAR)
    KW = 1.0 / (128.0 * CBAR)

    with tile.TileContext(nc) as tc:
        with (
            tc.tile_pool(name="const", bufs=1) as constp,
            tc.tile_pool(name="xbf", bufs=8) as xbfp,
            tc.tile_pool(name="xtg", bufs=3) as xtgp,
            tc.tile_pool(name="xs", bufs=4) as xsp,
            tc.tile_pool(name="mb", bufs=4) as mbp,
            tc.tile_pool(name="dbf", bufs=9) as dbfp,
            tc.tile_pool(name="st", bufs=4) as stp,
            tc.tile_pool(name="sm", bufs=4) as smp,
            tc.tile_pool(name="awt", bufs=2) as awtp,
            tc.tile_pool(name="osb", bufs=3) as osbp,
            tc.tile_pool(name="pstx", bufs=2, space="PSUM") as pstxp,
            tc.tile_pool(name="ps12", bufs=1, space="PSUM") as ps12p,
            tc.tile_pool(name="ps3", bufs=2, space="PSUM") as ps3p,
            tc.tile_pool(name="psaw", bufs=1, space="PSUM") as psawp,
        ):
            # ---------------- constants ----------------
            ident = constp.tile([128, 128], f32)
            masks.make_identity(nc, ident[:])
            ident_bf = constp.tile([128, 128], bf16, tag="identbf")
            nc.gpsimd.tensor_copy(ident_bf[:], ident[:])

            v2 = constp.tile([128, F], f32)
            nc.sync.dma_start(v2[0:64, :], v_d[:, :])
            nc.sync.dma_start(v2[64:128, :], v_d[:, :])
            v2bf = constp.tile([128, F], bf16, tag="v2bf")
            nc.gpsimd.tensor_copy(v2bf[:], v2[:])

            wt = {}
            for di in range(2):
                for ej in range(2):
                    t = constp.tile([128, 128], f32, tag=f"wt{di}{ej}")
                    nc.sync.dma_start(
                        t[:], w_d[di * 128:(di + 1) * 128, ej * 128:(ej + 1) * 128]
                    )
                    wt[di, ej] = t

            # load q naturally (contiguous DMA) and transpose on the PE —
            # a transposed DMA access pattern on q generates 8K 4-byte
            # descriptors and stalls the sync queue for ~25us.
            qn = constp.tile([64, E], f32, tag="qn")
            nc.sync.dma_start(qn[:], q_d[:, :])
            psq = ps12p.tile([128, 512], f32, tag="ps12")
            qtin = []
            for di in range(2):
                nc.tensor.transpose(
                    psq[:, di * 64:(di + 1) * 64],
                    qn[:, di * 128:(di + 1) * 128],
                    ident[0:64, 0:64],
                )
                t = constp.tile([128, N], f32, tag=f"qtin{di}")
                nc.scalar.copy(t[:], psq[:, di * 64:(di + 1) * 64])
                qtin.append(t)

            # qtilde^T = W.T-contract: qt[e, n] = sum_d W[d, e] q[n, d], then * SCALE
            qt_bf = []
            for ej in range(2):
                ps = ps12p.tile([128, 512], f32, tag="ps12")
                for di in range(2):
                    nc.tensor.matmul(
                        ps[:, 0:N], wt[di, ej][:], qtin[di][:],
                        start=(di == 0), stop=(di == 1),
                    )
                t = constp.tile([128, N], bf16, tag=f"qtbf{ej}")
                nc.scalar.mul(t[:], ps[:, 0:N], SCALE)
                qt_bf.append(t)

            # ---------------- per-group emit helpers ----------------
            def emit_front(g):
                """Generator: loads + PE transposes + stage-12 for group g.
                Returns (xbf, xs_t) via StopIteration."""
                b0 = g * C
                xbf = xbfp.tile([128, C, E], bf16, tag="xbf")
                for q in range(C // 8):
                    s0 = q * 8
                    nc.gpsimd.dma_start(
                        xbf[0:64, s0:s0 + 8, :],
                        x_d[b0 + s0:b0 + s0 + 8, :, :].transpose([1, 0, 2]),
                    )
                    nc.gpsimd.dma_start(
                        xbf[64:128, s0:s0 + 8, :],
                        x_d[HALF + b0 + s0:HALF + b0 + s0 + 8, :, :]
                        .transpose([1, 0, 2]),
                    )
                    yield

                # x^T via PE transposes (the DMA x-bar path serializes at
                # ~1.2us per 128x128 block on the HWDGE engine -- measured
                # 3.6x worse end-to-end).
                xtg = xtgp.tile([128, C, 2, 128], bf16, tag="xtg")
                for c4 in range(C // 4):
                    pst = pstxp.tile([128, 4, 2, 128], bf16, tag="pstx")
                    for k in range(4):
                        c = c4 * 4 + k
                        for ec in range(2):
                            nc.tensor.transpose(
                                pst[:, k, ec, :],
                                xbf[:, c, ec * 128:(ec + 1) * 128],
                                ident_bf[:],
                            )
                    # balance PSUM evac load: scalar is the busiest engine,
                    # vector has headroom -- split the copies between them.
                    if c4 == 0:
                        nc.vector.tensor_copy(
                            xtg[:, c4 * 4:c4 * 4 + 4, :, :], pst[:]
                        )
                    else:
                        nc.scalar.copy(xtg[:, c4 * 4:c4 * 4 + 4, :, :], pst[:])
                    yield

                xs_t = xsp.tile([128, C, 72], bf16, tag="xs")
                xs3 = xs_t[:, :, 0:F]
                for blk in range(C // 8):
                    ps = ps12p.tile([128, 512], f32, tag="ps12")
                    psA = ps[0:64, :].rearrange("p (c f) -> p c f", f=64)
                    psB = ps[64:128, :].rearrange("p (c f) -> p c f", f=64)
                    c0 = blk * 8
                    for ec in range(2):
                        # one FD=512 matmul per row-half covering 8 pairs
                        # (strided 3D rhs AP); start=True only on the first
                        # MM of each row-half (clears has_written for the
                        # MM's rows across the whole bank).
                        nc.tensor.matmul(
                            psA, qt_bf[ec][:],
                            xtg[:, c0:c0 + 8, ec, 0:64],
                            start=(ec == 0), stop=(ec == 1),
                            tile_position=(0, 0),
                            skip_group_check=True,
                        )
                        nc.tensor.matmul(
                            psB, qt_bf[ec][:],
                            xtg[:, c0:c0 + 8, ec, 64:128],
                            start=(ec == 0), stop=(ec == 1),
                            tile_position=(0, 64),
                            skip_group_check=True,
                        )
                        yield
                    nc.vector.tensor_copy(
                        xs3[:, c0:c0 + 8, :],
                        ps[:].rearrange("p (c f) -> p c f", f=F),
                    )
                    yield
                return xbf, xs_t

            def emit_entmax(xs_t):
                """Generator: entmax Newton solve, bf16 data-path with tau
                duplicated into bf16 pairs so the broadcast TT ops run in
                the DVE 2x mode.  Moments via per-c bn_stats (fine-grained
                ops interleave across lanes better than coarse reduces)."""
                xs3 = xs_t[:, :, 0:F]
                xs4 = xs3.rearrange("p c (k two) -> p c k two", two=2)
                st = stp.tile([128, C, 6], f32, tag="st")

                def sl(k):
                    return st[:, :, k:k + 1]        # [128, C, 1]

                tau = smp.tile([128, C], f32, tag="tau")
                tauu = tau[:].unsqueeze(2)
                tau2 = smp.tile([128, C, 2], bf16, tag="tau2")
                tau4 = tau2[:].unsqueeze(2).broadcast_to([128, C, 32, 2])
                s2s = smp.tile([128, C], f32, tag="s2s")
                rcp = smp.tile([128, C], f32, tag="rcp")

                # ---- init moments + tau0 ----
                for c in range(C):
                    nc.vector.bn_stats(st[:, c, :], xs3[:, c, :])
                    yield
                msum = smp.tile([128, C], f32, tag="msum")
                wsum = smp.tile([128, C], f32, tag="wsum")
                nc.vector.tensor_add(msum[:].unsqueeze(2), sl(1), sl(4))
                yield
                nc.vector.tensor_add(wsum[:].unsqueeze(2), sl(2), sl(5))
                yield
                nc.vector.tensor_scalar(
                    out=msum[:], in0=msum[:], scalar1=0.5, scalar2=K0,
                    op0=Alu.mult, op1=Alu.subtract,
                )
                yield
                nc.vector.scalar_tensor_tensor(
                    out=tau[:], in0=wsum[:], scalar=KW, in1=msum[:],
                    op0=Alu.mult, op1=Alu.add,
                )
                yield
                nc.vector.tensor_copy(
                    tau2[:], tau[:].unsqueeze(2).broadcast_to([128, C, 2])
                )
                yield

                mb_t = mbp.tile([128, C, 72], bf16, tag="mb")
                mb3 = mb_t[:, :, 0:F]
                mb4 = mb3.rearrange("p c (k two) -> p c k two", two=2)

                a2 = smp.tile([128, C, 2], f32, tag="a2")
                u2 = smp.tile([128, C, 2], f32, tag="u2")
                s1m = smp.tile([128, C], f32, tag="s1m")

                # ---- one Newton iteration (moments via bn_stats) ----
                nc.vector.tensor_max(mb4, xs4, tau4)
                yield
                for c in range(C):
                    nc.vector.bn_stats(st[:, c, :], mb3[:, c, :])
                    yield
                nc.vector.tensor_sub(a2[:, :, 0:1], sl(1), tauu)
                yield
                nc.vector.tensor_sub(a2[:, :, 1:2], sl(4), tauu)
                yield
                nc.vector.tensor_mul(u2[:], a2[:], a2[:])
                yield
                nc.vector.scalar_tensor_tensor(
                    out=u2[:, :, 0:1], in0=u2[:, :, 0:1], scalar=32.0,
                    in1=sl(2), op0=Alu.mult, op1=Alu.add,
                )
                yield
                nc.vector.scalar_tensor_tensor(
                    out=u2[:, :, 1:2], in0=u2[:, :, 1:2], scalar=32.0,
                    in1=sl(5), op0=Alu.mult, op1=Alu.add,
                )
                yield
                nc.vector.tensor_reduce(
                    s2s[:], u2[:], axis=mybir.AxisListType.X, op=Alu.add,
                )
                yield
                nc.vector.tensor_reduce(
                    s1m[:], a2[:], axis=mybir.AxisListType.X, op=Alu.add,
                )
                yield
                nc.vector.reciprocal_approx_fast(rcp[:], s1m[:])
                yield
                nc.vector.tensor_scalar(
                    out=s2s[:], in0=s2s[:], scalar1=-1.0, scalar2=None,
                    op0=Alu.add,
                )
                yield
                nc.vector.tensor_mul(s2s[:], s2s[:], rcp[:])
                yield
                nc.vector.scalar_tensor_tensor(
                    out=tau[:], in0=s2s[:], scalar=1.0 / 64.0, in1=tau[:],
                    op0=Alu.mult, op1=Alu.add,
                )
                yield
                nc.vector.tensor_copy(
                    tau2[:], tau[:].unsqueeze(2).broadcast_to([128, C, 2])
                )
                yield

                # ---- final eval ----
                # No final normalization: the Newton update drives s2 -> 1
                # to second order (|s2-1| ~ 2e-3, below bf16 noise), so
                # aw = relu(Xs-tau)^2 * v directly.
                nc.vector.tensor_max(mb4, xs4, tau4)
                yield
                d_bf = dbfp.tile([128, C, F], bf16, tag="dbf")
                d4 = d_bf[:].rearrange("p c (k two) -> p c k two", two=2)
                nc.vector.tensor_sub(d4, mb4, tau4)
                yield
                s_bf = dbfp.tile([128, C, F], bf16, tag="sbf")
                nc.vector.tensor_mul(s_bf[:], d_bf[:], d_bf[:])
                yield
                t_bf = dbfp.tile([128, C, F], bf16, tag="tbf")
                nc.vector.tensor_mul(
                    t_bf[:], s_bf[:],
                    v2bf[:].unsqueeze(1).broadcast_to([128, C, F]),
                )
                yield
                return t_bf

            def emit_back(g, xbf, t_bf):
                """Generator: PE transposes of aw + stage-3 + Exp + stores."""
                b0 = g * C
                aw_t = t_bf[:].rearrange("p c f -> p (c f)")

                # aw^T: bf16 PE transposes + evac; one shift DMA/group
                awt_g = awtp.tile([128, C, 64], bf16, tag="awt")
                awt_tmp = awtp.tile([64, C, 64], bf16, tag="awt_tmp")
                for hb in range(2):
                    pst = psawp.tile([64, half, 128], bf16, tag="psaw")
                    for s in range(half):
                        c = hb * half + s
                        nc.tensor.transpose(
                            pst[:, s, :], aw_t[:, c * F:(c + 1) * F],
                            ident_bf[:],
                        )
                        if s % 4 == 3:
                            yield
                    nc.scalar.copy(
                        awt_g[0:64, hb * half:(hb + 1) * half, :],
                        pst[:, :, 0:64],
                    )
                    nc.scalar.copy(
                        awt_tmp[:, hb * half:(hb + 1) * half, :],
                        pst[:, :, 64:128],
                    )
                    yield
                nc.sync.dma_start(awt_g[64:128, :, :], awt_tmp[:])
                yield

                # stage-3 (bf16): out = exp(awt.T @ x); Exp batched FD=1024
                for hb in range(2):
                    osb = osbp.tile([128, half, E], bf16, tag="osb")
                    for qq in range(C // 8):
                        ps3 = ps3p.tile([128, 1024], f32, tag="ps3")
                        # A/B matmuls interleaved per pair: adjacent A and B
                        # run concurrently in disjoint PE quadrants
                        # ((0,0) vs (64,64)) -- grouping all-A-then-all-B
                        # loses that packing (measured +18us).
                        for si in range(4):
                            c = hb * half + qq * 4 + si
                            nc.tensor.matmul(
                                ps3[0:64, si * 256:(si + 1) * 256],
                                awt_g[0:64, c, :],
                                xbf[0:64, c, :],
                                start=True, stop=True,
                                tile_position=(0, 0),
                                skip_group_check=True,
                            )
                            nc.tensor.matmul(
                                ps3[64:128, si * 256:(si + 1) * 256],
                                awt_g[64:128, c, :],
                                xbf[64:128, c, :],
                                start=True, stop=True,
                                tile_position=(64, 64),
                                skip_group_check=True,
                            )
                        nc.scalar.activation(
                            osb[:, qq * 4:(qq + 1) * 4, :],
                            ps3[:].rearrange("p (c e) -> p c e", e=E),
                            ACTF.Exp,
                        )
                        yield
                    bA = b0 + hb * half
                    nc.sync.dma_start(
                        o_d[bA:bA + half, :, :].transpose([1, 0, 2]),
                        osb[0:64, :, :],
                    )
                    nc.sync.dma_start(
                        o_d[HALF + bA:HALF + bA + half, :, :]
                        .transpose([1, 0, 2]),
                        osb[64:128, :, :],
                    )
                    yield

            def drive(items, carry, conts):
                """Round-robin generators.  `items` = [(gen, key)] that must
                finish this epoch; `carry` = keyless generators kept running;
                `conts[key](result)` spawns a follow-on generator (appended,
                keyless).  Returns (results, leftover carry generators)."""
                res = {}
                must = {k for _, k in items}
                act = [[g, k] for g, k in items] + [[g, None] for g in carry]
                while must - res.keys():
                    for job in list(act):
                        gen, key = job
                        try:
                            next(gen)
                        except StopIteration as e:
                            if key is not None:
                                res[key] = e.value
                            act.remove(job)
                            if key in conts:
                                act.append([conts[key](e.value), None])
                return res, [g for g, _ in act]

            # 3-stage pipeline, LANES-wide: epoch k interleaves
            # front(k+1-lane-set) | entmax(k-lane-set) | leftover backs;
            # each back spawns the moment its entmax finishes.  Wider lanes
            # overlap more per-group serial dependency chains (the span is
            # latency-bound, not engine-throughput-bound).
            LANES = 2
            NS = NG // LANES
            res, carry = drive(
                [(emit_front(j), f'f{j}') for j in range(LANES)], [], {})
            fr = [res[f'f{j}'] for j in range(LANES)]
            for k in range(NS):
                items = [(emit_entmax(fr[j][1]), f'e{j}')
                         for j in range(LANES)]
                if k + 1 < NS:
                    items += [(emit_front(LANES * (k + 1) + j), f'f{j}')
                              for j in range(LANES)]
                conts = {
                    f'e{j}': (lambda r, xb=fr[j][0], g=LANES * k + j:
                              emit_back(g, xb, r))
                    for j in range(LANES)
                }
                res, carry = drive(items, carry, conts)
                if k + 1 < NS:
                    fr = [res[f'f{j}'] for j in range(LANES)]
            # drain the remaining backs
            drive([(g, i) for i, g in enumerate(carry)], [], {})
    if not nc.is_finalized():
        nc.finalize()
    return nc


_NC_CACHE = {}


def _get_program(B_loc, NG):
    key = (B_loc, NG)
    if key not in _NC_CACHE:
        _NC_CACHE[key] = build_program(B_loc, NG)
    return _NC_CACHE[key]


def kernel(**inputs):
    from concourse.bass_utils import run_bass_kernel_spmd

    x = np.ascontiguousarray(np.asarray(inputs["x"], dtype=np.float32))
    w = np.ascontiguousarray(np.asarray(inputs["bilinear_w"], dtype=np.float32))
    q = np.ascontiguousarray(np.asarray(inputs["query"], dtype=np.float32))
    v = np.ascontiguousarray(np.asarray(inputs["value"], dtype=np.float32))
    B = x.shape[0]
    B_loc = B // NCORES

    nc = _get_program(B_loc, 8)

    in_maps = []
    for core in range(NCORES):
        sh = x[core * B_loc:(core + 1) * B_loc]
        in_maps.append(
            {"x": np.ascontiguousarray(sh), "bilinear_w": w, "query": q, "value": v}
        )

    import os
    trace = bool(int(os.environ.get("KERNEL_TRACE", "0")))
    res = run_bass_kernel_spmd(
        nc, in_maps, core_ids=list(range(NCORES)), trace=trace,
        trace_cores=[0] if trace else None,
    )
    if trace:
        kernel.last_exec_time_ns = res.exec_time_ns
        kernel.last_trace = res.instructions_and_trace
    out = np.concatenate([r["out"] for r in res.results], axis=0)
    return out.astype(np.float32)


if __name__ == "__main__":
    # smoke-test the builder only
    nc = build_program(32, 2)
    print("build ok:", len(nc.inst_map), "instructions")
